# revision 30
# baseline (speedup 1.0000x reference)
"""LocalAttention (B=4, H=16, L=2048, D=64, R=256) Trainium2 kernel.

The reference mask `(j-i >= 2048) | (j-i <= 1792)` keeps only keys with
j - i >= 1793.  Consequences (verified numerically vs the reference):
  * queries i in [0, 254] attend to the key band j in [i+1793, 2047]
    (masked logits underflow to exactly 0 after exp in f32, like the
    reference's exp(-10000 - max)),
  * queries i in [255, 2047] have every key masked -> softmax is uniform
    -> output row = mean(v over L).

So per (b, h) head we compute:
  1. mean_v = (1/2048) * sum_l v[l, :]            -> rows 255..2047
  2. a 255x255 "triangular band" attention with
     Q = q[0:255], K = k[1793:2047], V = v[1793:2047]  -> rows 0..254

Sharding: 64 (b,h) pairs, 8 per NeuronCore (data+head parallel, no
cross-device comm).  Per core the host ships: transposed Q/K bands
(qkT), the V band with fused ones-columns (vbo, for the softmax
denominator), and the full v (for the mean).  Host work is layout
marshalling only (transpose/concat), no arithmetic.

DMA queues are spread across the three issue engines (SP-HWDGE,
ACT-HWDGE, gpsimd-SWDGE) since DMA is the critical path.

NOTE this walrus build rejects instructions with more than one attached
sync wait, so `_legalize_waits` splits them into single-wait NoOps.
"""

import numpy as np
from contextlib import ExitStack

import concourse.bass as bass
import concourse.mybir as mybir
import concourse.tile as tile
from concourse.bass_utils import run_bass_kernel_spmd

B, H, L, D = 4, 16, 2048, 64
BH = B * H            # 64 (b,h) pairs
NCORES = 8
PER = BH // NCORES    # 8 pairs per core
BAND = 256            # padded band (queries 0..255 / keys 1792..2047)
NQ = 255              # valid band queries (0..254)
JCH = 16              # v rows packed per partition in the mean tile

F32 = mybir.dt.float32
EXP = mybir.ActivationFunctionType.Exp
SCALE = 0.125         # 1/sqrt(D)


def _build_bass():
    nc = bass.Bass()
    qkT = nc.declare_dram_parameter("qkT", [PER, D, 2 * BAND], F32, isOutput=False)
    vbo = nc.declare_dram_parameter("vbo", [PER, 128, 2 * (D + 1)], F32,
                                    isOutput=False)
    # v in j-major layout: vm[p, d*16+j] = v[16p+j, d] (host marshalled)
    # so the per-d reduce over j is unit-stride on DVE
    vv = nc.declare_dram_parameter("vm", [PER, 128, JCH * D], F32, isOutput=False)
    out = nc.declare_dram_parameter("out", [PER, L, D], F32, isOutput=True)

    with tile.TileContext(nc) as tc:
        with ExitStack() as ctx:
            vpool = ctx.enter_context(tc.tile_pool(name="vpool", bufs=3))
            io = ctx.enter_context(tc.tile_pool(name="io", bufs=3))
            ep = ctx.enter_context(tc.tile_pool(name="ep", bufs=3))
            small = ctx.enter_context(tc.tile_pool(name="small", bufs=4))
            ps_st = ctx.enter_context(tc.tile_pool(name="ps_st", bufs=3, space="PSUM"))
            ps_u = ctx.enter_context(tc.tile_pool(name="ps_u", bufs=4, space="PSUM"))

            for ibh in range(PER):
                # ---------------- loads ----------------
                # full v, contiguous 512KB (partition p = rows 16p..16p+15),
                # on the SP HWDGE queue
                v_tile = vpool.tile([128, JCH * D], F32)
                nc.sync.dma_start(out=v_tile, in_=vv[ibh])
                # transposed Q|K band [D, 512] on the ACT HWDGE queue,
                # V band + ones [128, 130] on the SWDGE queue
                qk = io.tile([D, 2 * BAND], F32, tag="qk")
                nc.scalar.dma_start(out=qk, in_=qkT[ibh])
                vb = io.tile([128, 2 * (D + 1)], F32, tag="vb")
                nc.scalar.dma_start(out=vb, in_=vbo[ibh])

                # ---------------- mean(v) over L ----------------
                vsum = small.tile([128, D], F32)
                nc.vector.reduce_sum(
                    out=vsum[:, :, None],
                    in_=v_tile.rearrange("p (d j) -> p d j", j=JCH),
                    axis=mybir.AxisListType.X,
                )
                mean_ps = ps_u.tile([1, D], F32, tag="u")
                nc.tensor.matmul(mean_ps, lhsT=vb[:, D:D + 1], rhs=vsum,
                                 start=True, stop=True)
                mean_sb = small.tile([1, D], F32)
                nc.vector.tensor_scalar_mul(mean_sb, mean_ps, 1.0 / float(L))
                # broadcast mean row to out rows 255..2047 (replicated source)
                msb = mean_sb[:, :]
                mean_bc = bass.AP(
                    tensor=msb.tensor,
                    offset=msb.offset,
                    ap=[list(msb.ap[0]), [0, L - NQ], [1, D]],
                )
                nc.gpsimd.dma_start(out=out[ibh, NQ:L, :], in_=mean_bc)

                # ---------------- band attention ----------------
                # scores (keys on partitions, queries on free dim), both
                # key-chunks into one PSUM tile: cols 0:128 = (k0, q0),
                # cols 128:384 = (k1, q0|q1)
                st = ps_st.tile([128, 384], F32, tag="st")
                nc.tensor.matmul(st[:, 0:128], lhsT=qk[:, BAND:BAND + 128],
                                 rhs=qk[:, 0:128], start=True, stop=True)
                nc.tensor.matmul(st[:, 128:384], lhsT=qk[:, BAND + 128:2 * BAND],
                                 rhs=qk[:, 0:BAND], start=True, stop=True)

                # exp(score/sqrt(D)); no max-subtraction needed (|s| <= ~7)
                e = ep.tile([128, 384], F32)
                nc.scalar.activation(e, st, EXP, scale=SCALE)
                # mask on the idle gpsimd engine: zero the invalid entries
                # key chunk0 vs q chunk0: keep iff p - f - 1 >= 0 (f < p)
                nc.gpsimd.affine_select(
                    out=e[:, 0:128], in_=e[:, 0:128],
                    compare_op=mybir.AluOpType.is_ge,
                    fill=0.0, base=-1, channel_multiplier=1,
                    pattern=[[-1, 128]],
                )
                # key chunk1 vs q0|q1: keep iff p - f + 127 >= 0
                nc.gpsimd.affine_select(
                    out=e[:, 128:384], in_=e[:, 128:384],
                    compare_op=mybir.AluOpType.is_ge,
                    fill=0.0, base=127, channel_multiplier=1,
                    pattern=[[-1, BAND]],
                )

                # U = P^T V (+ denominator in column D via the ones column)
                u0 = ps_u.tile([128, D + 1], F32, tag="u")
                nc.tensor.matmul(u0, lhsT=e[:, 0:128], rhs=vb[:, 0:D + 1],
                                 start=True, stop=False)
                nc.tensor.matmul(u0, lhsT=e[:, 128:256], rhs=vb[:, D + 1:],
                                 start=False, stop=True)
                u1 = ps_u.tile([128, D + 1], F32, tag="u")
                nc.tensor.matmul(u1, lhsT=e[:, 256:384], rhs=vb[:, D + 1:],
                                 start=True, stop=True)

                # normalize rows and store the band output
                r0 = small.tile([128, 1], F32, tag="r")
                r1 = small.tile([128, 1], F32, tag="r")
                nc.vector.reciprocal(r0, u0[:, D:D + 1])
                # query row 255 (f=127 of chunk1) is fully masked -> den = 0;
                # keep it finite (the row is never stored)
                den1 = small.tile([128, 1], F32, tag="r")
                nc.vector.tensor_scalar_add(den1, u1[:, D:D + 1], 1e-20)
                nc.vector.reciprocal(r1, den1)
                ob0 = small.tile([128, D], F32, tag="ob")
                ob1 = small.tile([128, D], F32, tag="ob")
                nc.vector.tensor_scalar_mul(ob0, u0[:, 0:D], r0)
                nc.vector.tensor_scalar_mul(ob1, u1[:, 0:D], r1)
                nc.sync.dma_start(out=out[ibh, 0:128, :], in_=ob0)
                nc.scalar.dma_start(out=out[ibh, 128:NQ, :], in_=ob1[0:127, :])

    return nc


def _legalize_waits(nc):
    """This walrus build rejects instructions carrying more than one
    attached sync wait (per-struct slot limits, e.g. PE Matmult and the
    kernel-tail Drain).  Split every multi-wait instruction's waits into
    preceding single-wait NoOps on the same engine queue — same-queue
    ordering preserves semantics exactly."""
    n = 0
    for fn in nc.m.functions:
        for blk in fn.blocks:
            new_insts = []
            for inst in blk.instructions:
                si = inst.sync_info
                if si is not None and si.on_wait and len(si.on_wait) > 1:
                    for w in si.on_wait:
                        n += 1
                        new_insts.append(mybir.InstNoOp(
                            name=f"legwait-{n}",
                            engine=inst.engine,
                            ins=[], outs=[],
                            sync_info=mybir.SyncInfo(on_wait=[w], on_update=[]),
                            bass_nofuse=True,
                        ))
                    inst.sync_info = mybir.SyncInfo(
                        on_wait=[], on_update=list(si.on_update or []))
                new_insts.append(inst)
            blk.instructions[:] = new_insts


_NC = None
_LEGALIZED = False


def _get_nc(legalize=False):
    global _NC, _LEGALIZED
    if _NC is None:
        _NC = _build_bass()
    if legalize and not _LEGALIZED:
        # CoreSim chokes on the injected NoOps, so only legalize for the
        # HW compile path
        _legalize_waits(_NC)
        _LEGALIZED = True
    return _NC


def _make_in_maps(q, k, v):
    qf = np.asarray(q, dtype=np.float32).reshape(BH, L, D)
    kf = np.asarray(k, dtype=np.float32).reshape(BH, L, D)
    vf = np.asarray(v, dtype=np.float32).reshape(BH, L, D)
    # host-side layout marshalling (no arithmetic): transpose the Q/K
    # bands, pack the V band with ones-columns
    qkT = np.concatenate(
        [qf[:, 0:BAND, :].transpose(0, 2, 1),
         kf[:, L - BAND:L, :].transpose(0, 2, 1)], axis=2)
    qkT = np.ascontiguousarray(qkT)                      # [BH, D, 512]
    vband = vf[:, L - BAND:L, :].reshape(BH, 2, 128, D)  # [BH, 2, 128, 64]
    vbo = np.ones((BH, 128, 2 * (D + 1)), dtype=np.float32)
    vbo[:, :, 0:D] = vband[:, 0]
    vbo[:, :, D + 1:2 * D + 1] = vband[:, 1]
    in_maps = []
    for c in range(NCORES):
        s = slice(c * PER, (c + 1) * PER)
        in_maps.append({
            "qkT": qkT[s],
            "vbo": np.ascontiguousarray(vbo[s]),
            "vm": np.ascontiguousarray(
                vf[s].reshape(PER, 128, JCH, D).transpose(0, 1, 3, 2)
                .reshape(PER, 128, JCH * D)),
        })
    return in_maps


def _run(q, k, v, **kwargs):
    nc = _get_nc(legalize=True)
    in_maps = _make_in_maps(q, k, v)
    return run_bass_kernel_spmd(nc, in_maps, list(range(NCORES)), **kwargs)


def kernel(q, k, v):
    res = _run(q, k, v)
    outs = [res.results[c]["out"] for c in range(NCORES)]
    return np.concatenate(outs, axis=0).reshape(B, H, L, D)
